# revision 1
# baseline (speedup 1.0000x reference)
"""Trainium2 Bass kernel for: Conv3d(3,16,k=3,valid) + bias -> channel softmax
-> maxpool 4x4x4/4.  Input x [512,3,16,32,32] f32 -> out [512,16,3,7,7] f32.

Sharding: pure data parallel, batch 512 -> 8 cores x 64 samples.

Per-core algorithm (all shapes per core):
  Conv as banded-stationary matmul: output h-rows are processed in 4 strips
  (8,8,8,6 rows).  For strip t the stationary lhsT is [K, 128] where
  K = 3kw*3ci*Hl rows (Hl = 10 input h-rows; 8 for the last strip) and
  M = 128 = 8 h-slots x 16 couts.  kh is folded into the band structure of
  the stationary; kd is handled by 3 PSUM-accumulating matmuls with shifted
  rhs APs; kw is handled by 3 flat-shifted SBUF copies of the input rows.
  rhs free dims = (d_out 14, w_out 30) = 420 columns.
  Then: ACT exp(y+bias) -> e bf16; ones-blockdiag matmul -> S replicated to
  all 128 partitions; DVE fast reciprocal -> r; e*r -> p; strided max-reduces
  pool w (4) and d (4); DMA accum_op=max pools h across partitions.
  Host reassembles the tiny pooled output.
"""

import sys

if "/opt/trn_rl_repo" not in sys.path:
    sys.path.insert(0, "/opt/trn_rl_repo")

from contextlib import ExitStack

import numpy as np
import ml_dtypes

import concourse.bass as bass  # noqa: F401
import concourse.tile as tile
from concourse import bacc, mybir
from concourse.bass_utils import run_bass_kernel_spmd

N_CORES = 8
NS = 64                   # samples per core
CIN, COUT = 3, 16
D, H, W = 16, 32, 32
DO, HO, WO = 14, 30, 30   # conv output spatial dims
NCOL = DO * WO            # matmul free size (420)
SB = 16                   # samples per streaming block
NBLK = NS // SB
SBF = SB * D * W          # free elements per block (8192)
PD, PH, PW = 3, 7, 7      # pooled output dims
PU = PD * PW              # 21 pooled (d,w) elements per (sample, strip)

F32 = mybir.dt.float32
BF16 = mybir.dt.bfloat16
BF16_NP = ml_dtypes.bfloat16

_STRIPS = [(0, 10, 8), (8, 10, 8), (16, 10, 8), (24, 8, 6)]  # (h0, Hl, gmax)

_CACHE = {}


def _host_consts(w, b):
    """Precompute stationary matrices + bias vector on host."""
    w = np.asarray(w, np.float32)
    b = np.asarray(b, np.float32)

    # h-slot g sits at partition position bitrev(g) so that the two h-pool
    # windows {g0..3}, {g4..7} reduce to contiguous partition halves via two
    # fold steps (max of partition halves).
    pos = [0, 4, 2, 6, 1, 5, 3, 7]  # pos[g] = bitrev3(g)

    # K-row order (kw, hl, ci): matches xs built from x2's (h, ci) partition
    # layout by 3 contiguous-partition shifted copies (one per kw).
    def band(kd, hl_n, g_n):
        m = np.zeros((9 * hl_n, 128), np.float32)
        for kw in range(3):
            for ci in range(CIN):
                for hl in range(hl_n):
                    k = kw * 3 * hl_n + hl * CIN + ci
                    for g in range(g_n):
                        kh = hl - g
                        if 0 <= kh <= 2:
                            for c in range(COUT):
                                m[k, pos[g] * COUT + c] = w[c, ci, kd, kh, kw]
        return m.astype(BF16_NP)

    consts = {}
    for kd in range(3):
        consts[f"wba{kd}"] = band(kd, 10, 8)   # strips 0-2: K=90
        consts[f"wbb{kd}"] = band(kd, 8, 6)    # strip 3:   K=72
    ones = np.zeros((128, 128), np.float32)
    for g in range(8):
        ones[g * COUT:(g + 1) * COUT, g * COUT:(g + 1) * COUT] = 1.0
    consts["onesbd"] = ones.astype(BF16_NP)
    consts["bvec"] = np.tile(b, 8).reshape(128, 1).astype(np.float32)
    return consts


def _build_program(repeat=1):
    nc = bacc.Bacc("TRN2", target_bir_lowering=False, debug=False,
                   enable_asserts=True, num_devices=N_CORES)
    # x pre-transposed on host to [(ci h), (s d w)] with 2 pad cols.
    xr = nc.dram_tensor("xr", [96, NS * D * W + 2], F32,
                        kind="ExternalInput").ap()
    wba = [nc.dram_tensor(f"wba{kd}", [90, 128], BF16, kind="ExternalInput").ap()
           for kd in range(3)]
    wbb = [nc.dram_tensor(f"wbb{kd}", [72, 128], BF16, kind="ExternalInput").ap()
           for kd in range(3)]
    onesbd = nc.dram_tensor("onesbd", [128, 128], BF16, kind="ExternalInput").ap()
    bvec = nc.dram_tensor("bvec", [128, 1], F32, kind="ExternalInput").ap()
    outa = nc.dram_tensor("outa", [16, NS * 4 * PU], F32,
                          kind="ExternalOutput").ap()
    outb = nc.dram_tensor("outb", [16, NS * 3 * PU], F32,
                          kind="ExternalOutput").ap()

    with tile.TileContext(nc) as tc, ExitStack() as ctx:
        const = ctx.enter_context(tc.tile_pool(name="const", bufs=1))
        wba_sb = []
        wbb_sb = []
        for kd in range(3):
            t_ = const.tile([90, 128], BF16, tag=f"wba{kd}")
            nc.sync.dma_start(t_[:], wba[kd])
            wba_sb.append(t_)
            t_ = const.tile([72, 128], BF16, tag=f"wbb{kd}")
            nc.sync.dma_start(t_[:], wbb[kd])
            wbb_sb.append(t_)
        ones_sb = const.tile([128, 128], BF16, tag="onesbd")
        nc.sync.dma_start(ones_sb[:], onesbd)
        bvec_sb = const.tile([128, 1], F32, tag="bvec")
        nc.sync.dma_start(bvec_sb[:], bvec)

        mpool = ctx.enter_context(tc.tile_pool(name="m", bufs=1))
        m_buf = mpool.tile([128, NS * 4 * PU], BF16)      # (s, t, do, wo)

        xpool = ctx.enter_context(tc.tile_pool(name="x2", bufs=2))
        xspool = ctx.enter_context(tc.tile_pool(name="xs", bufs=3))
        py = ctx.enter_context(tc.tile_pool(name="py", bufs=2, space="PSUM"))
        ps = ctx.enter_context(tc.tile_pool(name="ps", bufs=2, space="PSUM"))
        epool = ctx.enter_context(tc.tile_pool(name="e", bufs=3))
        rpool = ctx.enter_context(tc.tile_pool(name="r", bufs=2))
        ppool = ctx.enter_context(tc.tile_pool(name="p", bufs=2))
        pwpool = ctx.enter_context(tc.tile_pool(name="pw", bufs=2))
        hpool = ctx.enter_context(tc.tile_pool(name="hm", bufs=1))

        for _rep in range(repeat):
            for blk in range(NBLK):
                # x2: [(ci h) 96, (s d w) 8192 + 2 pad]; contiguous slice load
                x2 = xpool.tile([96, SBF + 2], BF16, tag="x2")
                nc.gpsimd.dma_start(  # f32 -> bf16 cast in DMA
                    x2[:], xr[:, blk * SBF: blk * SBF + SBF + 2])

                for t, (h0, hl_n, g_n) in enumerate(_STRIPS):
                    K = 9 * hl_n
                    xs = xspool.tile([K, SBF], BF16, tag="xs")
                    # row (kw,hl,ci) = x2 row (h0+hl, ci) shifted left by kw.
                    # cols 30,31 of each (s,d) w-row are then stale for kw>0
                    # but the matmul rhs only ever reads w' 0..29.
                    for kw in range(3):
                        nc.sync.dma_start(
                            xs[3 * hl_n * kw: 3 * hl_n * (kw + 1), :],
                            x2[3 * h0: 3 * (h0 + hl_n), kw:kw + SBF])
                    xs4 = xs[:].rearrange("k (s d w) -> k s d w", s=SB, d=D)
                    wsel = wba_sb if t < 3 else wbb_sb
                    for s in range(SB):
                        y = py.tile([128, NCOL], F32, tag="y")
                        for kd in range(3):
                            rhs = xs4[:, s, kd:kd + DO, 0:WO]
                            nc.tensor.matmul(y[:], wsel[kd][:], rhs,
                                             start=(kd == 0), stop=(kd == 2))
                        et = epool.tile([128, NCOL], BF16, tag="e")
                        nc.scalar.activation(
                            et[:], y[:], mybir.ActivationFunctionType.Exp,
                            bias=bvec_sb[:])
                        srep = ps.tile([128, NCOL], F32, tag="s")
                        nc.tensor.matmul(srep[:], ones_sb[:], et[:],
                                         start=True, stop=True)
                        rrep = rpool.tile([128, NCOL], F32, tag="r")
                        nc.vector.reciprocal_approx_fast(rrep[:], srep[:])
                        p = ppool.tile([128, NCOL], BF16, tag="p")
                        nc.vector.tensor_mul(p[:], et[:], rrep[:])
                        # pool w: [128,(d,wo,wi)] -> [128,(d,wo)]
                        pw = pwpool.tile([128, DO * PW], BF16, tag="pw")
                        pv = p[:].rearrange("m (d w) -> m d w", d=DO)
                        pv = pv[:, :, 0:PW * 4].rearrange(
                            "m d (wo wi) -> m d wo wi", wi=4)
                        pwv = pw[:].rearrange("m (d wo) -> m d wo", d=DO)
                        nc.vector.tensor_reduce(
                            pwv, pv, axis=mybir.AxisListType.X,
                            op=mybir.AluOpType.max)
                        # pool d: [128,(do,di,wo)] -> m_buf slice [128,(do,wo)]
                        sg = blk * SB + s
                        pdv = pw[:, 0:PD * 4 * PW].rearrange(
                            "m (do di wo) -> m do wo di", di=4, wo=PW)
                        mslice = m_buf[:, (sg * 4 + t) * PU:(sg * 4 + t + 1) * PU]
                        nc.vector.tensor_reduce(
                            mslice.rearrange("m (do wo) -> m do wo", do=PD),
                            pdv, axis=mybir.AxisListType.X,
                            op=mybir.AluOpType.max)

            # h-pool across partitions: partition index = bitrev(g)*16+c, so
            # window A = {g0..3} and B = {g4..7} fall out of two fold-max
            # steps over partition halves (DMA align + DVE max).
            FU = NS * 4 * PU
            tmp1 = hpool.tile([64, FU], BF16, tag="tmp1")
            q1 = hpool.tile([64, FU], BF16, tag="q1")
            nc.sync.dma_start(tmp1[:], m_buf[64:128, :])
            nc.vector.tensor_max(q1[:], m_buf[0:64, :], tmp1[:])
            tmp2 = hpool.tile([32, FU], BF16, tag="tmp2")
            hm = hpool.tile([32, FU], BF16, tag="hm")
            nc.sync.dma_start(tmp2[:], q1[32:64, :])
            nc.vector.tensor_max(hm[:], q1[0:32, :], tmp2[:])
            # rows 0:16 = window A (hw=2t), rows 16:32 = window B (hw=2t+1,
            # valid t<3 only).  bf16 -> f32 cast on the way out.
            nc.gpsimd.dma_start(outa, hm[0:16, :])
            hm3 = hm[16:32, :].rearrange("c (s t u) -> c s t u", s=NS, t=4)
            ob3 = outb.rearrange("c (s t u) -> c s t u", s=NS, t=3)
            nc.gpsimd.dma_start(ob3, hm3[:, :, 0:3, :])

    nc.compile()
    return nc


def _get_program(repeat=1):
    key = ("prog", repeat)
    if key not in _CACHE:
        _CACHE[key] = _build_program(repeat)
    return _CACHE[key]


def kernel(x, w, b):
    x = np.asarray(x, np.float32)
    consts = _host_consts(w, b)
    nc = _get_program()
    in_maps = []
    for c in range(N_CORES):
        xs_ = x[c * NS:(c + 1) * NS]                       # [64,3,16,32,32]
        xrr = xs_.transpose(3, 1, 0, 2, 4).reshape(96, NS * D * W)
        xrr = np.concatenate(
            [xrr, np.zeros((96, 2), np.float32)], axis=1)  # pad 2 cols
        m = {"xr": np.ascontiguousarray(xrr)}
        m.update(consts)
        in_maps.append(m)
    import time
    t0 = time.time()
    res = run_bass_kernel_spmd(nc, in_maps, core_ids=list(range(N_CORES)))
    _CACHE["last_wall_s"] = time.time() - t0

    out = np.empty((N_CORES * NS, COUT, PD, PH, PW), np.float32)
    for c in range(N_CORES):
        oa = res.results[c]["outa"].reshape(16, NS, 4, PD, PW)
        ob = res.results[c]["outb"].reshape(16, NS, 3, PD, PW)
        s0 = c * NS
        for t in range(4):
            out[s0:s0 + NS, :, :, 2 * t, :] = oa[:, :, t].transpose(1, 0, 2, 3)
        for t in range(3):
            out[s0:s0 + NS, :, :, 2 * t + 1, :] = (
                ob[:, :, t].transpose(1, 0, 2, 3))
    return out



# revision 6
# speedup vs baseline: 3.7009x; 3.7009x over previous
"""Trainium2 Bass kernel for: Conv3d(3,16,k=3,valid) + bias -> channel softmax
-> maxpool 4x4x4/4.  Input x [512,3,16,32,32] f32 -> out [512,16,3,7,7] f32.

Sharding: pure data parallel, batch 512 -> 8 cores x 64 samples.

Wall-clock on this setup is dominated by the axon host<->device tunnel
(~140 MB/s through the jit path) plus per-call dispatch, so the host path is
engineered around that:
  - x ships as bf16 in its NATURAL [512,3,16,32,32] layout (100 MB instead of
    201 MB, no host transpose); the (ci,h)-partition gather happens on-device
    in the x2 load DMA.
  - all weight-derived stationaries + bias pack into ONE small [128,897] bf16
    input; outputs merge into ONE [16,9408] f32 tensor per core.
  - the shard_map jit is built ONCE and cached; per call we only cast x,
    pack consts, call the cached executable, and fetch one output array.

Per-core algorithm (all shapes per core):
  Conv as banded-stationary matmul: output h-rows are processed in 4 strips
  (8,8,8,6 rows).  For strip t the stationary lhsT is [K, 128] where
  K = 3kw*3ci*Hl rows (Hl = 10 input h-rows; 8 for the last strip) and
  M = 128 = 8 h-slots x 16 couts.  kh is folded into the band structure of
  the stationary; kd is handled by 3 PSUM-accumulating matmuls with shifted
  rhs APs; kw is handled by 3 flat-shifted SBUF copies of the input rows.
  rhs free dims = (d_out 14, w_out 30) = 420 columns.
  Then: ACT exp(y+bias) -> e bf16; ones-blockdiag matmul -> S replicated to
  all 128 partitions; DVE fast reciprocal -> r; e*r -> p; strided max-reduces
  pool w (4) and d (4); DMA accum_op=max pools h across partitions.
  Host reassembles the tiny pooled output.
"""

import sys

if "/opt/trn_rl_repo" not in sys.path:
    sys.path.insert(0, "/opt/trn_rl_repo")

from contextlib import ExitStack

import numpy as np
import ml_dtypes

import concourse.bass as bass  # noqa: F401
import concourse.tile as tile
from concourse import bacc, mybir

N_CORES = 8
NS = 64                   # samples per core
CIN, COUT = 3, 16
D, H, W = 16, 32, 32
DO, HO, WO = 14, 30, 30   # conv output spatial dims
NCOL = DO * WO            # matmul free size (420)
SB = 16                   # samples per streaming block
NBLK = NS // SB
SBF = SB * D * W          # free elements per block (8192)
PD, PH, PW = 3, 7, 7      # pooled output dims
PU = PD * PW              # 21 pooled (d,w) elements per (sample, strip)
CCOLS = 3 * 128 + 3 * 128 + 128 + 1   # packed consts: wba x3, wbb x3, ones, b

F32 = mybir.dt.float32
BF16 = mybir.dt.bfloat16
BF16_NP = ml_dtypes.bfloat16

_STRIPS = [(0, 10, 8), (8, 10, 8), (16, 10, 8), (24, 8, 6)]  # (h0, Hl, gmax)

_CACHE = {}


def _host_consts(w, b):
    """Pack stationary matrices + bias into one [128, CCOLS] bf16 array."""
    w = np.asarray(w, np.float32)
    b = np.asarray(b, np.float32)

    # h-slot g sits at partition position bitrev(g) so that the two h-pool
    # windows {g0..3}, {g4..7} reduce to contiguous partition halves via two
    # fold steps (max of partition halves).
    pos = [0, 4, 2, 6, 1, 5, 3, 7]  # pos[g] = bitrev3(g)

    # K-row order (kw, ci, hl): matches xs built from x2's (ci, h) partition
    # layout by 9 contiguous-partition shifted copies (one per kw, ci).
    def band(kd, hl_n, g_n):
        m = np.zeros((9 * hl_n, 128), np.float32)
        for kw in range(3):
            for ci in range(CIN):
                for hl in range(hl_n):
                    k = (kw * CIN + ci) * hl_n + hl
                    for g in range(g_n):
                        kh = hl - g
                        if 0 <= kh <= 2:
                            for c in range(COUT):
                                m[k, pos[g] * COUT + c] = w[c, ci, kd, kh, kw]
        return m

    cst = np.zeros((128, CCOLS), np.float32)
    for kd in range(3):
        cst[0:90, kd * 128:(kd + 1) * 128] = band(kd, 10, 8)
        cst[0:72, 384 + kd * 128:384 + (kd + 1) * 128] = band(kd, 8, 6)
    for g in range(8):
        cst[g * COUT:(g + 1) * COUT, 768 + g * COUT:768 + (g + 1) * COUT] = 1.0
    cst[:, 896] = np.tile(b, 8)
    return cst.astype(BF16_NP)


def _build_program():
    nc = bacc.Bacc("TRN2", target_bir_lowering=False, debug=False,
                   enable_asserts=True, num_devices=N_CORES)
    # x in natural per-core layout; the (ci,h) gather happens in the x2 DMA.
    xr = nc.dram_tensor("xr", [NS, CIN, D, H, W], BF16,
                        kind="ExternalInput").ap()
    cst = nc.dram_tensor("cst", [128, CCOLS], BF16, kind="ExternalInput").ap()
    # out free layout (s, j(7), u=21): j 0..3 = h-windows 0,2,4,6; j 4..6 =
    # h-windows 1,3,5.  Host unscrambles j -> hw.
    out = nc.dram_tensor("out", [16, NS * 7 * PU], F32,
                         kind="ExternalOutput").ap()

    with tile.TileContext(nc) as tc, ExitStack() as ctx:
        const = ctx.enter_context(tc.tile_pool(name="const", bufs=1))
        cst_sb = const.tile([128, CCOLS], BF16, tag="cst")
        nc.sync.dma_start(cst_sb[:], cst)
        wba_sb = [cst_sb[0:90, kd * 128:(kd + 1) * 128] for kd in range(3)]
        wbb_sb = [cst_sb[0:72, 384 + kd * 128:384 + (kd + 1) * 128]
                  for kd in range(3)]
        ones_sb = cst_sb[0:128, 768:896]
        bv32 = const.tile([128, 1], F32, tag="bv32")
        nc.scalar.copy(bv32[:], cst_sb[:, 896:897])  # bf16 -> f32 for ACT bias

        mpool = ctx.enter_context(tc.tile_pool(name="m", bufs=1))
        m_buf = mpool.tile([128, NS * 4 * PU], BF16)      # (s, t, do, wo)

        xpool = ctx.enter_context(tc.tile_pool(name="x2", bufs=2))
        xspool = ctx.enter_context(tc.tile_pool(name="xs", bufs=3))
        py = ctx.enter_context(tc.tile_pool(name="py", bufs=2, space="PSUM"))
        ps = ctx.enter_context(tc.tile_pool(name="ps", bufs=2, space="PSUM"))
        epool = ctx.enter_context(tc.tile_pool(name="e", bufs=3))
        rpool = ctx.enter_context(tc.tile_pool(name="r", bufs=2))
        ppool = ctx.enter_context(tc.tile_pool(name="p", bufs=2))
        pwpool = ctx.enter_context(tc.tile_pool(name="pw", bufs=2))
        hpool = ctx.enter_context(tc.tile_pool(name="hm", bufs=1))

        for blk in range(NBLK):
            # x2: [(ci h) 96, (s d w) 8192]; gathered from natural layout.
            # DMA APs cap at 3 dims, so gather per (ci, s): dst [32h, 512]
            # <- src (h, d, w) strided view of one sample-channel.
            x2 = xpool.tile([96, SBF], BF16, tag="x2")
            for ci in range(CIN):
                for s in range(SB):
                    src = xr[blk * SB + s:blk * SB + s + 1,
                             ci:ci + 1].rearrange(
                                 "s ci d h w -> h (s ci d) w")
                    nc.sync.dma_start(
                        x2[ci * H:(ci + 1) * H,
                           s * D * W:(s + 1) * D * W].rearrange(
                               "h (d w) -> h d w", d=D),
                        src)

            for t, (h0, hl_n, g_n) in enumerate(_STRIPS):
                K = 9 * hl_n
                xs = xspool.tile([K, SBF], BF16, tag="xs")
                # row (kw,ci,hl) = x2 row (ci, h0+hl) shifted left by kw.
                # Only cols 0..SBF-3 are ever consumed by the matmul rhs
                # (max flat col 8189), so width SBF-2 needs no source pad.
                for kw in range(3):
                    for ci in range(CIN):
                        nc.sync.dma_start(
                            xs[(kw * CIN + ci) * hl_n:
                               (kw * CIN + ci + 1) * hl_n, 0:SBF - 2],
                            x2[ci * H + h0: ci * H + h0 + hl_n,
                               kw:kw + SBF - 2])
                xs4 = xs[:].rearrange("k (s d w) -> k s d w", s=SB, d=D)
                wsel = wba_sb if t < 3 else wbb_sb
                for s in range(SB):
                    y = py.tile([128, NCOL], F32, tag="y")
                    for kd in range(3):
                        rhs = xs4[:, s, kd:kd + DO, 0:WO]
                        nc.tensor.matmul(y[:], wsel[kd], rhs,
                                         start=(kd == 0), stop=(kd == 2))
                    et = epool.tile([128, NCOL], BF16, tag="e")
                    nc.scalar.activation(
                        et[:], y[:], mybir.ActivationFunctionType.Exp,
                        bias=bv32[:])
                    srep = ps.tile([128, NCOL], F32, tag="s")
                    nc.tensor.matmul(srep[:], ones_sb, et[:],
                                     start=True, stop=True)
                    rrep = rpool.tile([128, NCOL], F32, tag="r")
                    nc.vector.reciprocal_approx_fast(rrep[:], srep[:])
                    p = ppool.tile([128, NCOL], BF16, tag="p")
                    nc.vector.tensor_mul(p[:], et[:], rrep[:])
                    # pool w: [128,(d,wo,wi)] -> [128,(d,wo)]
                    pw = pwpool.tile([128, DO * PW], BF16, tag="pw")
                    pv = p[:].rearrange("m (d w) -> m d w", d=DO)
                    pv = pv[:, :, 0:PW * 4].rearrange(
                        "m d (wo wi) -> m d wo wi", wi=4)
                    pwv = pw[:].rearrange("m (d wo) -> m d wo", d=DO)
                    nc.vector.tensor_reduce(
                        pwv, pv, axis=mybir.AxisListType.X,
                        op=mybir.AluOpType.max)
                    # pool d: [128,(do,di,wo)] -> m_buf slice [128,(do,wo)]
                    sg = blk * SB + s
                    pdv = pw[:, 0:PD * 4 * PW].rearrange(
                        "m (do di wo) -> m do wo di", di=4, wo=PW)
                    mslice = m_buf[:, (sg * 4 + t) * PU:(sg * 4 + t + 1) * PU]
                    nc.vector.tensor_reduce(
                        mslice.rearrange("m (do wo) -> m do wo", do=PD),
                        pdv, axis=mybir.AxisListType.X,
                        op=mybir.AluOpType.max)

        # h-pool across partitions: partition index = bitrev(g)*16+c, so
        # window A = {g0..3} and B = {g4..7} fall out of two fold-max
        # steps over partition halves (DMA align + DVE max).
        FU = NS * 4 * PU
        tmp1 = hpool.tile([64, FU], BF16, tag="tmp1")
        q1 = hpool.tile([64, FU], BF16, tag="q1")
        nc.sync.dma_start(tmp1[:], m_buf[64:128, :])
        nc.vector.tensor_max(q1[:], m_buf[0:64, :], tmp1[:])
        tmp2 = hpool.tile([32, FU], BF16, tag="tmp2")
        hm = hpool.tile([32, FU], BF16, tag="hm")
        nc.sync.dma_start(tmp2[:], q1[32:64, :])
        nc.vector.tensor_max(hm[:], q1[0:32, :], tmp2[:])
        # rows 0:16 = window A (hw=2t) -> j 0..3; rows 16:32 = window B
        # (hw=2t+1, valid t<3) -> j 4..6.  bf16 -> f32 cast on the way out.
        o4 = out.rearrange("c (s j u) -> c s j u", s=NS, j=7)
        hma = hm[0:16, :].rearrange("c (s t u) -> c s t u", s=NS, t=4)
        hmb = hm[16:32, :].rearrange("c (s t u) -> c s t u", s=NS, t=4)
        nc.gpsimd.dma_start(o4[:, :, 0:4, :], hma)
        nc.gpsimd.dma_start(o4[:, :, 4:7, :], hmb[:, :, 0:3, :])

    nc.compile()
    return nc


def _make_runner(nc):
    """Cached shard_map jit over the bass_exec custom call — the per-call
    replacement for run_bass_kernel_spmd (which re-traces and re-lowers the
    jit on every invocation)."""
    import jax
    from jax.sharding import Mesh, PartitionSpec
    from jax.experimental.shard_map import shard_map
    from concourse import bass2jax

    bass2jax.install_neuronx_cc_hook()

    partition_name = (nc.partition_id_tensor.name
                      if nc.partition_id_tensor else None)
    in_names, out_names, out_avals = [], [], []
    for alloc in nc.m.functions[0].allocations:
        if not isinstance(alloc, mybir.MemoryLocationSet):
            continue
        name = alloc.memorylocations[0].name
        if alloc.kind == "ExternalInput":
            if name != partition_name:
                in_names.append(name)
        elif alloc.kind == "ExternalOutput":
            shape = tuple(alloc.tensor_shape)
            dtype = mybir.dt.np(alloc.dtype)
            out_names.append(name)
            out_avals.append(jax.core.ShapedArray(shape, dtype))
    n_params = len(in_names)
    n_outs = len(out_avals)
    in_names = in_names + out_names
    if partition_name is not None:
        in_names.append(partition_name)
    donate = tuple(range(n_params, n_params + n_outs))

    def _body(*args):
        operands = list(args)
        if partition_name is not None:
            operands.append(bass2jax.partition_id_tensor())
        outs = bass2jax._bass_exec_p.bind(
            *operands,
            out_avals=tuple(out_avals),
            in_names=tuple(in_names),
            out_names=tuple(out_names),
            lowering_input_output_aliases=(),
            sim_require_finite=True,
            sim_require_nnan=True,
            nc=nc,
        )
        return tuple(outs)

    devices = jax.devices()[:N_CORES]
    mesh = Mesh(np.asarray(devices), ("core",))
    in_specs = (PartitionSpec("core"),) * (n_params + n_outs)
    out_specs = (PartitionSpec("core"),) * n_outs
    sharded = jax.jit(
        shard_map(_body, mesh=mesh, in_specs=in_specs, out_specs=out_specs,
                  check_rep=False),
        donate_argnums=donate, keep_unused=True)
    # donated zero output buffers, reused across calls (kernel writes every
    # output element, so their values never matter).
    zeros = [np.zeros((N_CORES * a.shape[0], *a.shape[1:]), a.dtype)
             for a in out_avals]
    return sharded, zeros


def _get_runtime():
    if "rt" not in _CACHE:
        nc = _build_program()
        _CACHE["rt"] = _make_runner(nc)
    return _CACHE["rt"]


# out j-slot -> h-window position: j=t holds hw=2t, j=4+t holds hw=2t+1.
_J_OF_HW = [0, 4, 1, 5, 2, 6, 3]


def kernel(x, w, b):
    fn, zeros = _get_runtime()
    import time
    t0 = time.time()
    xg = np.asarray(x).astype(BF16_NP)                 # [512,3,16,32,32]
    cst = _host_consts(w, b)                           # [128,897] bf16
    cst_g = np.ascontiguousarray(
        np.broadcast_to(cst, (N_CORES, 128, CCOLS))).reshape(
            N_CORES * 128, CCOLS)
    (outg,) = fn(xg, cst_g, zeros[0])
    o = np.asarray(outg).reshape(N_CORES, 16, NS, 7, PD, PW)
    _CACHE["last_wall_s"] = time.time() - t0
    # (core, c, s, j, pd, pw) -> reorder j to hw -> (n, c, pd, hw, pw)
    o = o[:, :, :, _J_OF_HW]
    return np.ascontiguousarray(
        o.transpose(0, 2, 1, 4, 3, 5)).reshape(N_CORES * NS, COUT, PD, PH, PW)


# revision 7
# speedup vs baseline: 4.5143x; 1.2198x over previous
"""Trainium2 Bass kernel for: Conv3d(3,16,k=3,valid) + bias -> channel softmax
-> maxpool 4x4x4/4.  Input x [512,3,16,32,32] f32 -> out [512,16,3,7,7] f32.

Sharding: pure data parallel, batch 512 -> 8 cores x 64 samples.

Wall-clock on this setup is dominated by the axon host<->device tunnel
(~140 MB/s through the jit path) plus per-call dispatch, so the host path is
engineered around that:
  - x ships as bf16 in its NATURAL [512,3,16,32,32] layout (100 MB instead of
    201 MB, no host transpose); the (ci,h)-partition gather happens on-device
    in the x2 load DMA.
  - all weight-derived stationaries + bias pack into ONE small [128,897] bf16
    input; outputs merge into ONE [16,9408] f32 tensor per core.
  - the shard_map jit is built ONCE and cached; per call we only cast x,
    pack consts, call the cached executable, and fetch one output array.

Per-core algorithm (all shapes per core):
  Conv as banded-stationary matmul: output h-rows are processed in 4 strips
  (8,8,8,6 rows).  For strip t the stationary lhsT is [K, 128] where
  K = 3kw*3ci*Hl rows (Hl = 10 input h-rows; 8 for the last strip) and
  M = 128 = 8 h-slots x 16 couts.  kh is folded into the band structure of
  the stationary; kd is handled by 3 PSUM-accumulating matmuls with shifted
  rhs APs; kw is handled by 3 flat-shifted SBUF copies of the input rows.
  rhs free dims = (d_out 14, w_out 30) = 420 columns.
  Then: ACT exp(y+bias) -> e bf16; ones-blockdiag matmul -> S replicated to
  all 128 partitions; DVE fast reciprocal -> r; e*r -> p; strided max-reduces
  pool w (4) and d (4); DMA accum_op=max pools h across partitions.
  Host reassembles the tiny pooled output.
"""

import sys

if "/opt/trn_rl_repo" not in sys.path:
    sys.path.insert(0, "/opt/trn_rl_repo")

from contextlib import ExitStack

import numpy as np
import ml_dtypes

import concourse.bass as bass  # noqa: F401
import concourse.tile as tile
from concourse import bacc, mybir

N_CORES = 8
NS = 64                   # samples per core
CIN, COUT = 3, 16
D, H, W = 16, 32, 32
DO, HO, WO = 14, 30, 30   # conv output spatial dims
NCOL = DO * WO            # matmul free size (420)
SB = 16                   # samples per streaming block
NBLK = NS // SB
SBF = SB * D * W          # free elements per block (8192)
PD, PH, PW = 3, 7, 7      # pooled output dims
PU = PD * PW              # 21 pooled (d,w) elements per (sample, strip)
CCOLS = 3 * 128 + 3 * 128 + 128 + 1   # packed consts: wba x3, wbb x3, ones, b

F32 = mybir.dt.float32
BF16 = mybir.dt.bfloat16
BF16_NP = ml_dtypes.bfloat16

_STRIPS = [(0, 10, 8), (8, 10, 8), (16, 10, 8), (24, 8, 6)]  # (h0, Hl, gmax)

_CACHE = {}


def _host_consts(w, b):
    """Pack stationary matrices + bias into one [128, CCOLS] bf16 array."""
    w = np.asarray(w, np.float32)
    b = np.asarray(b, np.float32)

    # h-slot g sits at partition position bitrev(g) so that the two h-pool
    # windows {g0..3}, {g4..7} reduce to contiguous partition halves via two
    # fold steps (max of partition halves).
    pos = [0, 4, 2, 6, 1, 5, 3, 7]  # pos[g] = bitrev3(g)

    # K-row order (kw, ci, hl): matches xs built from x2's (ci, h) partition
    # layout by 9 contiguous-partition shifted copies (one per kw, ci).
    def band(kd, hl_n, g_n):
        m = np.zeros((9 * hl_n, 128), np.float32)
        for kw in range(3):
            for ci in range(CIN):
                for hl in range(hl_n):
                    k = (kw * CIN + ci) * hl_n + hl
                    for g in range(g_n):
                        kh = hl - g
                        if 0 <= kh <= 2:
                            for c in range(COUT):
                                m[k, pos[g] * COUT + c] = w[c, ci, kd, kh, kw]
        return m

    cst = np.zeros((128, CCOLS), np.float32)
    for kd in range(3):
        cst[0:90, kd * 128:(kd + 1) * 128] = band(kd, 10, 8)
        cst[0:72, 384 + kd * 128:384 + (kd + 1) * 128] = band(kd, 8, 6)
    for g in range(8):
        cst[g * COUT:(g + 1) * COUT, 768 + g * COUT:768 + (g + 1) * COUT] = 1.0
    cst[:, 896] = np.tile(b, 8)
    return cst.astype(BF16_NP)


def _build_program():
    nc = bacc.Bacc("TRN2", target_bir_lowering=False, debug=False,
                   enable_asserts=True, num_devices=N_CORES)
    # x in natural per-core layout; the (ci,h) gather happens in the x2 DMA.
    xr = nc.dram_tensor("xr", [NS, CIN, D, H, W], BF16,
                        kind="ExternalInput").ap()
    cst = nc.dram_tensor("cst", [128, CCOLS], BF16, kind="ExternalInput").ap()
    # out free layout (s, j(7), u=21): j 0..3 = h-windows 0,2,4,6; j 4..6 =
    # h-windows 1,3,5.  Host unscrambles j -> hw.
    out = nc.dram_tensor("out", [16, NS * 7 * PU], BF16,
                         kind="ExternalOutput").ap()

    with tile.TileContext(nc) as tc, ExitStack() as ctx:
        const = ctx.enter_context(tc.tile_pool(name="const", bufs=1))
        cst_sb = const.tile([128, CCOLS], BF16, tag="cst")
        nc.sync.dma_start(cst_sb[:], cst)
        wba_sb = [cst_sb[0:90, kd * 128:(kd + 1) * 128] for kd in range(3)]
        wbb_sb = [cst_sb[0:72, 384 + kd * 128:384 + (kd + 1) * 128]
                  for kd in range(3)]
        ones_sb = cst_sb[0:128, 768:896]
        bv32 = const.tile([128, 1], F32, tag="bv32")
        nc.scalar.copy(bv32[:], cst_sb[:, 896:897])  # bf16 -> f32 for ACT bias

        mpool = ctx.enter_context(tc.tile_pool(name="m", bufs=1))
        m_buf = mpool.tile([128, NS * 4 * PU], BF16)      # (s, t, do, wo)

        xpool = ctx.enter_context(tc.tile_pool(name="x2", bufs=2))
        xspool = ctx.enter_context(tc.tile_pool(name="xs", bufs=3))
        py = ctx.enter_context(tc.tile_pool(name="py", bufs=2, space="PSUM"))
        ps = ctx.enter_context(tc.tile_pool(name="ps", bufs=2, space="PSUM"))
        epool = ctx.enter_context(tc.tile_pool(name="e", bufs=3))
        rpool = ctx.enter_context(tc.tile_pool(name="r", bufs=2))
        ppool = ctx.enter_context(tc.tile_pool(name="p", bufs=2))
        pwpool = ctx.enter_context(tc.tile_pool(name="pw", bufs=2))
        hpool = ctx.enter_context(tc.tile_pool(name="hm", bufs=1))

        for blk in range(NBLK):
            # x2: [(ci h) 96, (s d w) 8192]; gathered from natural layout.
            # DMA APs cap at 3 dims, so gather per (ci, s): dst [32h, 512]
            # <- src (h, d, w) strided view of one sample-channel.
            x2 = xpool.tile([96, SBF], BF16, tag="x2")
            for ci in range(CIN):
                for s in range(SB):
                    src = xr[blk * SB + s:blk * SB + s + 1,
                             ci:ci + 1].rearrange(
                                 "s ci d h w -> h (s ci d) w")
                    nc.sync.dma_start(
                        x2[ci * H:(ci + 1) * H,
                           s * D * W:(s + 1) * D * W].rearrange(
                               "h (d w) -> h d w", d=D),
                        src)

            for t, (h0, hl_n, g_n) in enumerate(_STRIPS):
                K = 9 * hl_n
                xs = xspool.tile([K, SBF], BF16, tag="xs")
                # row (kw,ci,hl) = x2 row (ci, h0+hl) shifted left by kw.
                # Only cols 0..SBF-3 are ever consumed by the matmul rhs
                # (max flat col 8189), so width SBF-2 needs no source pad.
                for kw in range(3):
                    for ci in range(CIN):
                        nc.sync.dma_start(
                            xs[(kw * CIN + ci) * hl_n:
                               (kw * CIN + ci + 1) * hl_n, 0:SBF - 2],
                            x2[ci * H + h0: ci * H + h0 + hl_n,
                               kw:kw + SBF - 2])
                xs4 = xs[:].rearrange("k (s d w) -> k s d w", s=SB, d=D)
                wsel = wba_sb if t < 3 else wbb_sb
                for s in range(SB):
                    y = py.tile([128, NCOL], F32, tag="y")
                    for kd in range(3):
                        rhs = xs4[:, s, kd:kd + DO, 0:WO]
                        nc.tensor.matmul(y[:], wsel[kd], rhs,
                                         start=(kd == 0), stop=(kd == 2))
                    et = epool.tile([128, NCOL], BF16, tag="e")
                    nc.scalar.activation(
                        et[:], y[:], mybir.ActivationFunctionType.Exp,
                        bias=bv32[:])
                    srep = ps.tile([128, NCOL], F32, tag="s")
                    nc.tensor.matmul(srep[:], ones_sb, et[:],
                                     start=True, stop=True)
                    rrep = rpool.tile([128, NCOL], F32, tag="r")
                    nc.vector.reciprocal_approx_fast(rrep[:], srep[:])
                    p = ppool.tile([128, NCOL], BF16, tag="p")
                    nc.vector.tensor_mul(p[:], et[:], rrep[:])
                    # pool w: [128,(d,wo,wi)] -> [128,(d,wo)]
                    pw = pwpool.tile([128, DO * PW], BF16, tag="pw")
                    pv = p[:].rearrange("m (d w) -> m d w", d=DO)
                    pv = pv[:, :, 0:PW * 4].rearrange(
                        "m d (wo wi) -> m d wo wi", wi=4)
                    pwv = pw[:].rearrange("m (d wo) -> m d wo", d=DO)
                    nc.vector.tensor_reduce(
                        pwv, pv, axis=mybir.AxisListType.X,
                        op=mybir.AluOpType.max)
                    # pool d: [128,(do,di,wo)] -> m_buf slice [128,(do,wo)]
                    sg = blk * SB + s
                    pdv = pw[:, 0:PD * 4 * PW].rearrange(
                        "m (do di wo) -> m do wo di", di=4, wo=PW)
                    mslice = m_buf[:, (sg * 4 + t) * PU:(sg * 4 + t + 1) * PU]
                    nc.vector.tensor_reduce(
                        mslice.rearrange("m (do wo) -> m do wo", do=PD),
                        pdv, axis=mybir.AxisListType.X,
                        op=mybir.AluOpType.max)

        # h-pool across partitions: partition index = bitrev(g)*16+c, so
        # window A = {g0..3} and B = {g4..7} fall out of two fold-max
        # steps over partition halves (DMA align + DVE max).
        FU = NS * 4 * PU
        tmp1 = hpool.tile([64, FU], BF16, tag="tmp1")
        q1 = hpool.tile([64, FU], BF16, tag="q1")
        nc.sync.dma_start(tmp1[:], m_buf[64:128, :])
        nc.vector.tensor_max(q1[:], m_buf[0:64, :], tmp1[:])
        tmp2 = hpool.tile([32, FU], BF16, tag="tmp2")
        hm = hpool.tile([32, FU], BF16, tag="hm")
        nc.sync.dma_start(tmp2[:], q1[32:64, :])
        nc.vector.tensor_max(hm[:], q1[0:32, :], tmp2[:])
        # rows 0:16 = window A (hw=2t) -> j 0..3; rows 16:32 = window B
        # (hw=2t+1, valid t<3) -> j 4..6.  Stays bf16 (values are already
        # bf16-quantized); host casts to f32 during reassembly.
        o4 = out.rearrange("c (s j u) -> c s j u", s=NS, j=7)
        hma = hm[0:16, :].rearrange("c (s t u) -> c s t u", s=NS, t=4)
        hmb = hm[16:32, :].rearrange("c (s t u) -> c s t u", s=NS, t=4)
        nc.gpsimd.dma_start(o4[:, :, 0:4, :], hma)
        nc.gpsimd.dma_start(o4[:, :, 4:7, :], hmb[:, :, 0:3, :])

    nc.compile()
    return nc


def _make_runner(nc):
    """Cached shard_map jit over the bass_exec custom call — the per-call
    replacement for run_bass_kernel_spmd (which re-traces and re-lowers the
    jit on every invocation)."""
    import jax
    from jax.sharding import Mesh, PartitionSpec
    from jax.experimental.shard_map import shard_map
    from concourse import bass2jax

    bass2jax.install_neuronx_cc_hook()

    partition_name = (nc.partition_id_tensor.name
                      if nc.partition_id_tensor else None)
    in_names, out_names, out_avals = [], [], []
    for alloc in nc.m.functions[0].allocations:
        if not isinstance(alloc, mybir.MemoryLocationSet):
            continue
        name = alloc.memorylocations[0].name
        if alloc.kind == "ExternalInput":
            if name != partition_name:
                in_names.append(name)
        elif alloc.kind == "ExternalOutput":
            shape = tuple(alloc.tensor_shape)
            dtype = mybir.dt.np(alloc.dtype)
            out_names.append(name)
            out_avals.append(jax.core.ShapedArray(shape, dtype))
    n_params = len(in_names)
    n_outs = len(out_avals)
    in_names = in_names + out_names
    if partition_name is not None:
        in_names.append(partition_name)
    donate = tuple(range(n_params, n_params + n_outs))

    def _body(*args):
        operands = list(args)
        if partition_name is not None:
            operands.append(bass2jax.partition_id_tensor())
        outs = bass2jax._bass_exec_p.bind(
            *operands,
            out_avals=tuple(out_avals),
            in_names=tuple(in_names),
            out_names=tuple(out_names),
            lowering_input_output_aliases=(),
            sim_require_finite=True,
            sim_require_nnan=True,
            nc=nc,
        )
        return tuple(outs)

    devices = jax.devices()[:N_CORES]
    mesh = Mesh(np.asarray(devices), ("core",))
    in_specs = (PartitionSpec("core"),) * (n_params + n_outs)
    out_specs = (PartitionSpec("core"),) * n_outs
    sharded = jax.jit(
        shard_map(_body, mesh=mesh, in_specs=in_specs, out_specs=out_specs,
                  check_rep=False),
        donate_argnums=donate, keep_unused=True)
    # donated zero output buffers, reused across calls (kernel writes every
    # output element, so their values never matter).
    zeros = [np.zeros((N_CORES * a.shape[0], *a.shape[1:]), a.dtype)
             for a in out_avals]
    return sharded, zeros


def _get_runtime():
    if "rt" not in _CACHE:
        nc = _build_program()
        _CACHE["rt"] = _make_runner(nc)
    return _CACHE["rt"]


# out j-slot -> h-window position: j=t holds hw=2t, j=4+t holds hw=2t+1.
_J_OF_HW = [0, 4, 1, 5, 2, 6, 3]


def kernel(x, w, b):
    fn, zeros = _get_runtime()
    import time
    t0 = time.time()
    xg = np.asarray(x).astype(BF16_NP)                 # [512,3,16,32,32]
    cst = _host_consts(w, b)                           # [128,897] bf16
    cst_g = np.ascontiguousarray(
        np.broadcast_to(cst, (N_CORES, 128, CCOLS))).reshape(
            N_CORES * 128, CCOLS)
    (outg,) = fn(xg, cst_g, zeros[0])
    o = np.asarray(outg).astype(np.float32).reshape(N_CORES, 16, NS, 7, PD, PW)
    _CACHE["last_wall_s"] = time.time() - t0
    # (core, c, s, j, pd, pw) -> reorder j to hw -> (n, c, pd, hw, pw)
    o = o[:, :, :, _J_OF_HW]
    return np.ascontiguousarray(
        o.transpose(0, 2, 1, 4, 3, 5)).reshape(N_CORES * NS, COUT, PD, PH, PW)
